# revision 15
# baseline (speedup 1.0000x reference)
"""Trainium2 Bass kernel for nn_BertEmbedding 'bissect' pooling head.

Reference computation (per batch b, token t):
    hs   = hidden_states[1:13]                 # drop embedding layer -> 12 layers
    v    = mean_n hs[n,b,t,:]                  # (768,)
    s_n  = v . hs[n,b,t,:]                     # 12 scores
    p    = softmax(s)                          # over the 12 layers
    final= sum_n p_n * hs[n,b,t,:]             # (768,)
    pooled[b,:] = max_t final[b,t,:]*mask[b,t]
    logits = pooled @ W.T + b                  # (b, 2)

Sharding: pure data parallelism over batch (32 batches -> 4 per core x 8 cores).

Architecture (v3, HW-measured engine balance):
  Measured DMA floor for the 75.5 MB/core of f32 loads: ~115us
  (~656 GB/s/core).  Constraints probed through real codegen: STT(+accum)
  is DVE-only (Pool rejects it); Pool does tensor_tensor/copy (incl.
  broadcast APs and f32->f16 casts); ACT does activation-class only.
  Design keeps every engine at or under the DMA floor:
  - f32->fp16 casts on ACT (idle otherwise; a few per unit moved to Pool)
    feed fp16 tiles to PE (fp16 matmul) and DVE (2x fp16 rate).
  - u = sum_n hs_n: PEU layers via identity-matmul PSUM accumulation on
    PE, the rest as fp16 tensor_adds on DVE/Pool; one fused combine
    (vec-half + PSUM half -> fp16 u) on DVE.
  - scores_n = sum_h (hs_n/12)*u: 12 fp16 STT+accum per subtile on DVE.
  - softmax batched per 256-token unit: one negated 3D max-reduce for
    both subtiles, per-subtile exp (bias=-max) with fused denominator
    accum, one 2-wide reciprocal.  No pscale: 1/denom folds into the
    token-max mask scalar.
  - diag(p_n): ONE broadcast-AP tensor_mul per subtile on Pool
    ([128,12,128] = id (x) exps) instead of 12 per-layer scalar muls.
  - final = sum_n p_n*hs_n: fp16 diag-matmul PSUM accumulation on PE.
  - masked token-max fused on DVE (scalar = mask*recip); cross-partition
    max via PE transpose + DVE free-dim reduce; tiny linear via
    ones-matmul.
"""

import os
import sys
from contextlib import ExitStack

import numpy as np

for _p in ("/opt/trn_rl_repo",):
    if _p not in sys.path:
        sys.path.insert(0, _p)

import concourse.bacc as bacc
import concourse.mybir as mybir
import concourse.tile as tile
from concourse.bass_utils import run_bass_kernel_spmd

F32 = mybir.dt.float32
F16 = mybir.dt.float16
AX = mybir.AxisListType
OP = mybir.AluOpType
ACT = mybir.ActivationFunctionType

NCORES = 8
L = 12          # layers used (hidden_states[1:13])
BFULL, TSEQ, H = 32, 512, 768
B = BFULL // NCORES            # batches per core = 4
NL = 2                         # num labels
HC = H // 128                  # h chunks of 128 = 6
X = 2                          # 128-token subtiles per DMA unit
TOK = 128 * X                  # tokens per unit = 256
NUNIT = TSEQ // TOK            # units per batch = 2
NEG_INF = -3.0e38

# layers of u accumulated on the PE (rest: fp16 tensor_adds on DVE/Pool)
PEU = int(os.environ.get("K_PEU", "6"))
# of the (12-PEU-1) vec-half u adds, how many run on Pool (rest DVE)
UADD_POOL = int(os.environ.get("K_UADD_POOL", "0"))
# f32->fp16 casts per unit on Pool (rest ACT)
CAST_POOL = int(os.environ.get("K_CAST_POOL", "2"))
# diag-build engine: P=Pool, D=DVE
DIAG_ENG = os.environ.get("K_DIAG_ENG", "P")
# pipeline truncation for HW bisection (timing only): 1=loads 2=+cast
# 3=+u 4=+scores 5=+softmax/diag 6=+final 7=full(default)
STAGE = int(os.environ.get("K_STAGE", "7"))

chunks = [(0, 512), (512, 256)]


def _build_nc(reps=1):
    nc = bacc.Bacc("TRN2", target_bir_lowering=False, debug=False,
                   num_devices=NCORES)
    hs_d = nc.declare_dram_parameter("hs", [L, B, TSEQ, H], F32, isOutput=False)
    mask_d = nc.declare_dram_parameter("mask", [B, TSEQ], F32, isOutput=False)
    wres_d = nc.declare_dram_parameter("wres", [128, NL * HC], F32, isOutput=False)
    bres_d = nc.declare_dram_parameter("bres", [1, B * NL], F32, isOutput=False)
    id32_d = nc.declare_dram_parameter("id32", [128, 128], F32, isOutput=False)
    id16_d = nc.declare_dram_parameter("id16", [128, 128], F16, isOutput=False)
    out_d = nc.declare_dram_parameter("out", [1, B * NL], F32, isOutput=True)

    with tile.TileContext(nc) as tc:
        with ExitStack() as ctx:
            _body(ctx, tc, nc, hs_d, mask_d, wres_d, bres_d, id32_d, id16_d,
                  out_d, reps)
    nc.compile()
    return nc


def _body(ctx, tc, nc, hs_d, mask_d, wres_d, bres_d, id32_d, id16_d, out_d,
          reps=1):
    singles = ctx.enter_context(tc.tile_pool(name="singles", bufs=1))
    hs_pool = ctx.enter_context(tc.tile_pool(name="hs", bufs=2))
    work = ctx.enter_context(tc.tile_pool(name="work", bufs=2))
    small = ctx.enter_context(tc.tile_pool(name="small", bufs=4))
    batchp = ctx.enter_context(tc.tile_pool(name="batchp", bufs=2))
    psum_u = ctx.enter_context(tc.tile_pool(name="psum_u", bufs=1, space="PSUM"))
    psum_f = ctx.enter_context(tc.tile_pool(name="psum_f", bufs=2, space="PSUM"))
    psum_tr = ctx.enter_context(tc.tile_pool(name="psum_tr", bufs=1, space="PSUM"))

    id32 = singles.tile([128, 128], F32)
    nc.sync.dma_start(out=id32, in_=id32_d[:, :])
    id16 = singles.tile([128, 128], F16)
    nc.sync.dma_start(out=id16, in_=id16_d[:, :])
    wres = singles.tile([128, NL * HC], F32)
    nc.sync.dma_start(out=wres, in_=wres_d[:, :])
    bres = singles.tile([1, B * NL], F32)
    nc.sync.dma_start(out=bres, in_=bres_d[:, :])
    ones = singles.tile([128, 1], F32)
    nc.vector.memset(ones, 1.0)
    partials = singles.tile([128, B * NL], F32)
    if STAGE < 7:
        nc.vector.memset(partials, 0.0)
    logits_sb = singles.tile([1, B * NL], F32)

    def batch_sweep():
        for bb in range(B):
            _batch(tc, nc, hs_d, mask_d, hs_pool, work, small, batchp,
                   psum_u, psum_f, psum_tr, id32, id16, wres, partials, bb)

    if reps == 1:
        batch_sweep()
    else:
        with tc.For_i(0, reps, 1):
            batch_sweep()

    # ---- reduce partials over partitions with a ones-matmul; add bias ----
    lg_ps = psum_tr.tile([1, B * NL], F32, tag="tr")
    nc.tensor.matmul(lg_ps, ones, partials, start=True, stop=True)
    nc.vector.tensor_add(logits_sb, lg_ps, bres)
    nc.sync.dma_start(out=out_d[:, :], in_=logits_sb)


def _batch(tc, nc, hs_d, mask_d, hs_pool, work, small, batchp, psum_u,
           psum_f, psum_tr, id32, id16, wres, partials, bb):
    maxacc = batchp.tile([128, H], F32, tag="maxacc")
    nc.gpsimd.memset(maxacc, NEG_INF)

    for hh in range(NUNIT):
        # ---- load 12 layer tiles (f32 staging), cast to fp16 ----
        hs = []
        for n in range(L):
            t32 = hs_pool.tile([128, X, H], F32, tag=f"hs32_{n}", bufs=1)
            nc.sync.dma_start(
                out=t32,
                in_=hs_d[n, bb, hh * TOK:(hh + 1) * TOK, :].rearrange(
                    "(x p) h -> p x h", p=128))
            t = hs_pool.tile([128, X, H], F16, tag=f"hs{n}", bufs=2)
            if STAGE >= 2:
                if n >= L - CAST_POOL:
                    nc.gpsimd.tensor_copy(t, t32)
                else:
                    nc.scalar.copy(t, t32)
            hs.append(t)
        msk = small.tile([128, X], F32, tag="msk")
        nc.sync.dma_start(
            out=msk,
            in_=mask_d[bb, hh * TOK:(hh + 1) * TOK].rearrange("(x p) -> p x", p=128))
        if STAGE < 3:
            continue

        # ---- vector half of u: fp16 adds over layers PEU..11 ----
        nvec = L - PEU
        uv = None
        if nvec == 1:
            uv = hs[PEU]
        elif nvec >= 2:
            uv = work.tile([128, X, H], F16, tag="uv", bufs=2)
            vec_layers = list(range(PEU, L))
            npool = min(UADD_POOL, nvec - 1)
            if npool > 0:
                uvP = work.tile([128, X, H], F16, tag="uvP", bufs=1)
                pl = vec_layers[-(npool + 1):]
                nc.gpsimd.tensor_add(uvP, hs[pl[0]], hs[pl[1]])
                for n in pl[2:]:
                    nc.gpsimd.tensor_add(uvP, uvP, hs[n])
                dl = vec_layers[:-(npool + 1)]
            else:
                uvP = None
                dl = vec_layers
            if len(dl) == 1:
                if uvP is None:
                    uv = hs[dl[0]]
                else:
                    nc.vector.tensor_add(uv, hs[dl[0]], uvP)
            else:
                nc.vector.tensor_add(uv, hs[dl[0]], hs[dl[1]])
                for n in dl[2:]:
                    nc.vector.tensor_add(uv, uv, hs[n])
                if uvP is not None:
                    nc.vector.tensor_add(uv, uv, uvP)

        scores = small.tile([128, X, L], F32, tag="scores")
        for x in range(X):
            # ---- PE half of u (identity accumulation) + fused combine ----
            if PEU > 0:
                u_ps = psum_u.tile([128, H], F32, tag="u")
                for c0, cw in chunks:
                    for n in range(PEU):
                        nc.tensor.matmul(
                            u_ps[:, c0:c0 + cw], id16,
                            hs[n][:, x, c0:c0 + cw],
                            start=(n == 0), stop=(n == PEU - 1))
                u = work.tile([128, H], F16, tag="u", bufs=2)
                if uv is None:
                    nc.vector.tensor_copy(u, u_ps)
                else:
                    nc.vector.tensor_add(u, uv[:, x], u_ps)
            else:
                u = uv[:, x]
            if STAGE < 4:
                continue

            # ---- scores_n = sum_h (hs_n/12)*u  (fp16 STT + accum, DVE) ----
            scrD = work.tile([128, H], F16, tag="scrD", bufs=1)
            for n in range(L):
                nc.vector.scalar_tensor_tensor(
                    out=scrD, in0=hs[n][:, x], scalar=1.0 / L,
                    in1=u, op0=OP.mult, op1=OP.mult,
                    accum_out=scores[:, x, n:n + 1])
            if STAGE < 5:
                continue

            # ---- softmax over the 12 layers (batched; no pscale) ----
            if x == X - 1:
                negmx = small.tile([128, X], F32, tag="negmx")
                nc.vector.tensor_reduce(out=negmx, in_=scores, axis=AX.X,
                                        op=OP.max, negate=True)
                exps = small.tile([128, X, L], F32, tag="exps")
                denom = small.tile([128, X], F32, tag="denom")
                for x2 in range(X):
                    nc.scalar.activation(out=exps[:, x2], in_=scores[:, x2],
                                         func=ACT.Exp,
                                         bias=negmx[:, x2:x2 + 1], scale=1.0,
                                         accum_out=denom[:, x2:x2 + 1])
                recip = small.tile([128, X], F32, tag="recip")
                nc.vector.reciprocal(recip, denom)
                mr = small.tile([128, X], F32, tag="mr")
                nc.vector.tensor_mul(mr, msk, recip)

        if STAGE < 5:
            continue

        for x in range(X):
            # ---- diag(exps_n) built in one broadcast op ----
            dg = work.tile([128, L, 128], F16, tag="dg", bufs=2)
            idb = id16.unsqueeze(1).broadcast_to([128, L, 128])
            eb = exps[:, x].unsqueeze(2).broadcast_to([128, L, 128])
            if DIAG_ENG == "P":
                nc.gpsimd.tensor_mul(dg, idb, eb)
            else:
                nc.vector.tensor_mul(dg, idb, eb)
            if STAGE < 6:
                continue

            # ---- final = sum_n exps_n*hs_n  (PE diag accumulation) ----
            fin_ps = psum_f.tile([128, H], F32, tag="fin")
            for c0, cw in chunks:
                for n in range(L):
                    nc.tensor.matmul(
                        fin_ps[:, c0:c0 + cw], dg[:, n],
                        hs[n][:, x, c0:c0 + cw],
                        start=(n == 0), stop=(n == L - 1))
            if STAGE < 7:
                continue

            # ---- masked running max over tokens (scalar = mask/denom) ----
            nc.vector.scalar_tensor_tensor(
                out=maxacc, in0=fin_ps, scalar=mr[:, x:x + 1], in1=maxacc,
                op0=OP.mult, op1=OP.max)

    if STAGE < 7:
        return
    # ---- pooled[b] = cross-partition max via PE transpose ----
    pooled = batchp.tile([128, HC], F32, tag="pooled")
    for c in range(HC):
        ptr = psum_tr.tile([128, 128], F32, tag="tr")
        nc.tensor.transpose(ptr, maxacc[:, c * 128:(c + 1) * 128], id32)
        nc.vector.tensor_reduce(out=pooled[:, c:c + 1], in_=ptr,
                                axis=AX.X, op=OP.max)

    # ---- logits partials: sum_h pooled*W per label ----
    sc6 = small.tile([128, HC], F32, tag="sc6")
    for l in range(NL):
        nc.vector.scalar_tensor_tensor(
            out=sc6, in0=pooled, scalar=1.0,
            in1=wres[:, l * HC:(l + 1) * HC], op0=OP.mult, op1=OP.mult,
            accum_out=partials[:, bb * NL + l:bb * NL + l + 1])


_NC_CACHE = None


def _get_nc():
    global _NC_CACHE
    if _NC_CACHE is None:
        _NC_CACHE = _build_nc()
    return _NC_CACHE


def kernel(hidden_states, mask, W, b):
    hidden_states = np.asarray(hidden_states, dtype=np.float32)
    mask = np.asarray(mask, dtype=np.float32)
    W = np.asarray(W, dtype=np.float32)
    b = np.asarray(b, dtype=np.float32)

    nc = _get_nc()

    # wres[p, l*HC+c] = W[l, c*128+p]
    wres = np.ascontiguousarray(
        W.reshape(NL, HC, 128).transpose(2, 0, 1).reshape(128, NL * HC))
    bres = np.ascontiguousarray(np.tile(b, B)[None, :])
    id32 = np.eye(128, dtype=np.float32)
    id16 = np.eye(128, dtype=np.float16)

    in_maps = []
    for ci in range(NCORES):
        in_maps.append({
            "hs": np.ascontiguousarray(hidden_states[1:, ci * B:(ci + 1) * B]),
            "mask": np.ascontiguousarray(mask[ci * B:(ci + 1) * B]),
            "wres": wres,
            "bres": bres,
            "id32": id32,
            "id16": id16,
        })

    res = run_bass_kernel_spmd(nc, in_maps, list(range(NCORES)))
    out = np.concatenate(
        [res.results[i]["out"].reshape(B, NL) for i in range(NCORES)], axis=0)
    return out


if __name__ == "__main__":
    rng = np.random.default_rng(0)
    hs = rng.standard_normal((13, BFULL, TSEQ, H), dtype=np.float32)
    mask = np.ones((BFULL, TSEQ), dtype=np.float32)
    W = rng.standard_normal((NL, H), dtype=np.float32) * 0.02
    b = np.zeros((NL,), dtype=np.float32)
    out = kernel(hidden_states=hs, mask=mask, W=W, b=b)
    print(out)
